# revision 7
# baseline (speedup 1.0000x reference)
"""Causal self-attention (Q=K=V=x, unscaled) on 8 trn2 NeuronCores.

x: [8, 2048, 512] f32. Data-parallel over batch: core b computes batch
element b entirely on-chip:
  S = x @ x.T   causal lower triangle only; f32r matmuls at full PE rate,
                chunks accumulate in PSUM and never round-trip raw scores
                through SBUF
  P = softmax(S) row-wise: causal mask added in-PSUM (DVE), per-chunk
                partial maxes (DVE), exp reads PSUM directly (ACT) with
                fused per-chunk row-sums
  out = P @ x   P tiles PE-transposed, f32r matmuls, normalization fused
                into the PSUM->SBUF output copy (ACT scale)
"""

import numpy as np

import concourse.bass as bass
import concourse.mybir as mybir
import concourse.tile as tile
from concourse import bacc
from concourse.bass_utils import run_bass_kernel_spmd
from concourse.masks import make_causal_mask, make_identity

B, S, D = 8, 2048, 512
P = 128
NQ = S // P  # 16 q-blocks of 128 rows
ND = D // P  # 4 contraction chunks of 128
CW = 512  # score chunk width (one PSUM bank of f32)
F32 = mybir.dt.float32
F32R = mybir.dt.float32r
MASK_VAL = -1e30


def _emit(nc: bass.Bass, reps: int = 1):
    x_d = nc.dram_tensor("x", [S, D], F32, kind="ExternalInput").ap()
    o_d = nc.dram_tensor("out", [S, D], F32, kind="ExternalOutput").ap()

    with tile.TileContext(nc) as tc:
        with (
            tc.tile_pool(name="const", bufs=1) as cpool,
            tc.tile_pool(name="xn", bufs=NQ) as xn_pool,
            tc.tile_pool(name="xt", bufs=ND) as xt_pool,
            tc.tile_pool(name="pstrip", bufs=2) as sc_pool,
            tc.tile_pool(name="pts", bufs=3) as pt_pool,
            tc.tile_pool(name="ob", bufs=2) as o_pool,
            tc.tile_pool(name="stat", bufs=2) as st_pool,
            tc.tile_pool(name="ps_sc", bufs=4, space="PSUM") as ps_sc,
            tc.tile_pool(name="ps_tp", bufs=2, space="PSUM") as ps_tp,
            tc.tile_pool(name="ps_pv", bufs=2, space="PSUM") as ps_pv,
        ):
            ident = cpool.tile([P, P], F32, tag="ident")
            make_identity(nc, ident[:])
            cmask = cpool.tile([P, P], F32, tag="cmask")
            make_causal_mask(nc, cmask[:], mask_val=MASK_VAL)

            if reps > 1:
                # benchmarking only: repeat the whole body in a HW loop
                import contextlib

                loop_cm = tc.For_i(
                    0, reps, 1, hint_engines=(mybir.EngineType.PE,)
                )
            else:
                import contextlib

                loop_cm = contextlib.nullcontext()
            with loop_cm:
                _emit_body(nc, tc, x_d, o_d, ident, cmask,
                           xn_pool, xt_pool, sc_pool, pt_pool, o_pool,
                           st_pool, ps_sc, ps_tp, ps_pv)


def _emit_body(nc, tc, x_d, o_d, ident, cmask, xn_pool, xt_pool, sc_pool,
               pt_pool, o_pool, st_pool, ps_sc, ps_tp, ps_pv):
            # x natural layout: 16 tiles [t=128, d=512]; xnr is the
            # f32r-rounded copy that feeds the P@x matmul as rhs
            xn = []
            xnr = []
            for ti in range(NQ):
                t = xn_pool.tile([P, D], F32, tag="xn", name=f"xn{ti}")
                nc.sync.dma_start(t[:], x_d[ti * P : (ti + 1) * P, :])
                xn.append(t)
                tr = xn_pool.tile([P, D], F32R, tag="xnr", name=f"xnr{ti}")
                nc.vector.tensor_copy(tr[:], t[:])
                xnr.append(tr)

            # x transposed: 4 tiles [d=128, t=2048] (both operands of x@x.T)
            xt = [
                xt_pool.tile([P, S], F32R, tag="xt", name=f"xt{dk}")
                for dk in range(ND)
            ]

            def emit_setup_group(tg):
                # Fill xt[:][:, tg*512:(tg+1)*512] by PE-transposing
                # x natural tiles tg*4..tg*4+3, one [128,128] block each.
                for dk in range(ND):
                    tp = ps_tp.tile([P, CW], F32, tag="tp")
                    for j in range(4):
                        ti = tg * 4 + j
                        nc.tensor.transpose(
                            tp[:, j * P : (j + 1) * P],
                            xn[ti][:, dk * P : (dk + 1) * P],
                            ident[:],
                        )
                    nc.vector.tensor_copy(
                        xt[dk][:, tg * CW : (tg + 1) * CW], tp[:]
                    )

            # Software pipeline: stage s emits scores+softmax for q-block s
            # and transposes+PV for q-block s-1, so DVE/ACT softmax of one
            # block overlaps PE matmuls of the next.
            state = [None] * NQ
            for step in range(NQ + 1):
                if step < NQ:
                    qi = step
                    if qi % 4 == 0:
                        emit_setup_group(qi // 4)
                    width = (qi + 1) * P
                    nfull, rem = divmod(width, CW)
                    # f32r needs a moving dim >= 256 for full rate
                    widths = [CW] * nfull + ([max(rem, 256)] if rem else [])
                    nch = len(widths)
                    pstrip = sc_pool.tile([P, S], F32, tag="pstrip")
                    pmax = st_pool.tile([P, ND], F32, tag="pmax")
                    chunks = []
                    for c, cw in enumerate(widths):
                        ps = ps_sc.tile([P, CW], F32, tag="ps", name=f"ps{qi}_{c}")
                        for dk in range(ND):
                            nc.tensor.matmul(
                                ps[:, :cw],
                                xt[dk][:, qi * P : (qi + 1) * P],
                                xt[dk][:, c * CW : c * CW + cw],
                                start=(dk == 0),
                                stop=(dk == ND - 1),
                            )
                        lo = c * CW
                        hi = min(width, lo + cw)
                        if hi > qi * P:
                            # chunk holds the diagonal 128x128 tile:
                            # apply the causal mask in place in PSUM
                            doff = qi * P - lo
                            nc.vector.tensor_add(
                                ps[:, doff : doff + P],
                                ps[:, doff : doff + P],
                                cmask[:],
                            )
                        nc.vector.reduce_max(
                            pmax[:, c : c + 1],
                            ps[:, : hi - lo],
                            axis=mybir.AxisListType.X,
                        )
                        chunks.append((ps, lo, hi))
                    nmax = st_pool.tile([P, 1], F32, tag="nmax")
                    nc.vector.reduce_max(
                        nmax[:],
                        pmax[:, :nch],
                        axis=mybir.AxisListType.X,
                        negate=True,
                    )
                    psums = st_pool.tile([P, ND], F32, tag="psums")
                    for c, (ps, lo, hi) in enumerate(chunks):
                        nc.scalar.activation(
                            pstrip[:, lo:hi],
                            ps[:, : hi - lo],
                            mybir.ActivationFunctionType.Exp,
                            bias=nmax[:],
                            scale=1.0,
                            accum_out=psums[:, c : c + 1],
                        )
                    rsum = st_pool.tile([P, 1], F32, tag="rsum")
                    nc.vector.reduce_sum(
                        rsum[:], psums[:, :nch], axis=mybir.AxisListType.X
                    )
                    rcp = st_pool.tile([P, 1], F32, tag="rcp")
                    nc.vector.reciprocal(rcp[:], rsum[:])
                    state[qi] = (pstrip, rcp)

                if step >= 1:
                    qi2 = step - 1
                    pstrip, rcp = state[qi2]
                    state[qi2] = None
                    ntile = qi2 + 1
                    pv = ps_pv.tile([P, D], F32, tag="pv")
                    for g0 in range(0, ntile, 4):
                        gn = min(4, ntile - g0)
                        tp = ps_tp.tile([P, CW], F32, tag="tp")
                        for j in range(gn):
                            ti = g0 + j
                            nc.tensor.transpose(
                                tp[:, j * P : (j + 1) * P],
                                pstrip[:, ti * P : (ti + 1) * P],
                                ident[:],
                            )
                        pts = pt_pool.tile([P, CW], F32R, tag="pts")
                        nc.vector.tensor_copy(pts[:, : gn * P], tp[:, : gn * P])
                        for j in range(gn):
                            ti = g0 + j
                            nc.tensor.matmul(
                                pv[:],
                                pts[:, j * P : (j + 1) * P],
                                xnr[ti][:],
                                start=(ti == 0),
                                stop=(ti == ntile - 1),
                            )
                    ob = o_pool.tile([P, D], F32, tag="ob")
                    nc.scalar.activation(
                        ob[:],
                        pv[:],
                        mybir.ActivationFunctionType.Copy,
                        bias=0.0,
                        scale=rcp[:],
                    )
                    nc.sync.dma_start(o_d[qi2 * P : (qi2 + 1) * P, :], ob[:])


_COMPILED = None


def _get_compiled():
    global _COMPILED
    if _COMPILED is None:
        nc = bacc.Bacc("TRN2", target_bir_lowering=False, debug=False)
        _emit(nc)
        nc.compile()
        _COMPILED = nc
    return _COMPILED


def kernel(x: np.ndarray) -> np.ndarray:
    assert x.shape == (B, S, D), x.shape
    nc = _get_compiled()
    in_maps = [
        {"x": np.ascontiguousarray(x[b], dtype=np.float32)} for b in range(B)
    ]
    res = run_bass_kernel_spmd(nc, in_maps, core_ids=list(range(B)))
    return np.stack([res.results[b]["out"] for b in range(B)], axis=0)
